# revision 17
# baseline (speedup 1.0000x reference)
"""Trainium2 Bass kernel for DepthwiseSeparableConv3d (inference).

Problem: x[2,48,48,48,64] -> dw3x3x3 depthwise + BN + ReLU -> 1x1x1 conv
(64->128) + BN + ReLU -> z[2,48,48,48,128], all f32.

Strategy (8 NeuronCores, data-parallel over (b,d) slabs, 12 slabs/core):
 - Host pre-pads D/H/W and folds the BN scales into the conv weights;
   BN biases ride along in the ReLU stage (ScalarE activation bias /
   DVE tensor_scalar add+max), so the device does matmul+ReLU+DMA only.
 - Depthwise conv on TensorE as a parity-folded 2D block-Toeplitz:
   K = 128 partitions = (4 ch x 8 w-in x 4 h-parity),
   M = 48 partitions  = (4 ch x 6 w-out x 2 h-parity).
   The 3 w-taps AND the 3 h-taps live in the Toeplitz stationary
   (partition (ci,wi,hw) holds column hp=2j+hw of the padded image, so
   every dy tap is reachable without a free-dim shift); only the 3
   dz-taps are PSUM-accumulated pumps.  9 -> 3 pumps per position vs a
   1-D Toeplitz: 1.5x fewer PE rows.
 - Two groups share each PSUM bank via PE column tiling
   (tile_position (0,0)/(0,64)); ReLU+bias alternates ScalarE/DVE.
 - The (ci,wo,p)->channel regroup DMA has 4-partition destinations and
   is split across the SP and ACT HWDGE rings.
 - Serial phases: all DW first (uninterrupted matmul stream keeps the
   PE at the top p-state), then the PW sweep.  A burst of dummy
   matmuls during the input-DMA wait pre-warms the PE clock.
 - Everything on-chip is fp16; output z is fp16, widened on host.
"""

import sys

for _p in ("/opt/trn_rl_repo", "/opt/pypackages"):
    if _p not in sys.path:
        sys.path.insert(0, _p)

import numpy as np

import concourse.bass as bass
import concourse.tile as tile
from concourse import bacc, mybir
from concourse.bass_utils import run_bass_kernel_spmd

# ----- problem constants (hardcoded per spec) -----
B, D, H, W, C, F = 2, 48, 48, 48, 64, 128
EPS = 1e-3
N_CORES = 8
DPC = (B * D) // N_CORES          # 12 d-slabs per core
DI = DPC + 2                      # 14 with halo
CG = 4                            # channels per group
NG = C // CG                      # 16 groups
NK = NG // 2                      # 8 supergroups (2 groups PSUM-stacked)
WT = 8                            # w tiles
WO = 6                            # w outputs per tile
WI = 8                            # w inputs per tile
NJ = 24                           # h-pairs
KP = CG * WI * 4                  # 128 partitions (ci, wi, hw)
MP = CG * WO * 2                  # 48 real outputs (ci, wo, p)
MPAD = 64                         # stationary cols padded to a PE col-tile
NCOL = DPC * NJ                   # 288 columns per pump
NPOS = WT * WO * 2 * DPC * NJ     # 27648 positions per core
NQ = 18                           # PW macro-chunks
QP = NPOS // NQ                   # 1536 positions per macro-chunk
QC = QP // 4                      # 384 per pump
WARMUP_MM = 48                    # dummy matmuls to pre-ramp the PE clock

F16 = mybir.dt.float16
F32 = mybir.dt.float32

_COMPILED = None


def _build_bass():
    nc = bacc.Bacc("TRN2", target_bir_lowering=False, debug=False,
                   num_devices=N_CORES)

    xt_d = nc.dram_tensor("xt", [NG, KP, WT, DI, NJ], F16,
                          kind="ExternalInput").ap()
    wt_d = nc.dram_tensor("wt", [KP, NG, 3, MPAD], F16,
                          kind="ExternalInput").ap()
    pw_d = nc.dram_tensor("pwk", [C, F], F16, kind="ExternalInput").ap()
    sb1_d = nc.dram_tensor("sb1", [128, NK], F32, kind="ExternalInput").ap()
    sb2_d = nc.dram_tensor("sb2", [F, 1], F32, kind="ExternalInput").ap()
    z_d = nc.dram_tensor("z", [F, NPOS], F16, kind="ExternalOutput").ap()
    y_d = nc.dram_tensor("ydram", [C, 16 * WT * DPC * NJ], F16,
                         kind="Internal").ap()

    RELU = mybir.ActivationFunctionType.Relu
    ADD = mybir.AluOpType.add
    MAX = mybir.AluOpType.max

    with tile.TileContext(nc) as tc:
        with (
            tc.tile_pool(name="consts", bufs=1) as consts,
            tc.tile_pool(name="xg", bufs=NG) as xpool,
            tc.tile_pool(name="ya", bufs=6) as yapool,
            tc.tile_pool(name="Ybig", bufs=1) as Ypool,
            tc.tile_pool(name="zt", bufs=3) as zpool,
        ):
            wt_sb = consts.tile([KP, NG, 3, MPAD], F16)
            pw_sb = consts.tile([C, F], F16)
            sb1_sb = consts.tile([128, NK], F32)
            sb2_sb = consts.tile([F, 1], F32)
            Y = Ypool.tile([C, 16, WT, DPC, NJ], F16)
            Yf = Y.rearrange("c q t d j -> c (q t d j)")
            xg = [xpool.tile([KP, WT, DI, NJ], F16, tag="xg", name=f"xg{g}")
                  for g in range(NG)]

            # Need-ordered input stream.  Only a few xg DMAs may be in
            # flight at once: descriptors of concurrent DMAs round-robin
            # across the DMA engines, so a deep queue makes every tile
            # finish late; the rest are issued from inside the k-loop.
            def load_xg(g):
                nc.sync.dma_start(xg[g][:, 0:4], xt_d[g, :, 0:4])
                nc.sync.dma_start(xg[g][:, 4:8], xt_d[g, :, 4:8])

            nc.sync.dma_start(wt_sb[:, 0:2], wt_d[:, 0:2])
            load_xg(0)
            load_xg(1)
            nc.sync.dma_start(sb1_sb[:], sb1_d[:])
            load_xg(2)
            load_xg(3)
            nc.sync.dma_start(wt_sb[:, 2:NG], wt_d[:, 2:NG])
            nc.sync.dma_start(pw_sb[:], pw_d[:])
            nc.sync.dma_start(sb2_sb[:], sb2_d[:])

            with tc.tile_pool(name="psdw", bufs=1, space="PSUM") as pdw:
                # p-state warmup: pump the PE on already-loaded weights while
                # the first xg DMAs are in flight; results are never read.
                if WARMUP_MM:
                    wps = pdw.tile([128, 2, 512], F32, tag="tq0", name="warm")
                    wrhs = wt_sb[:, 0:2, :, :].rearrange("k a b m -> k (a b m)")
                    for i in range(WARMUP_MM):
                        nc.tensor.matmul(wps[0:MPAD, i % 2, 0:NCOL],
                                         wt_sb[:, 0, 0, :], wrhs[:, 0:NCOL],
                                         start=True, stop=True,
                                         skip_group_check=True)

                for k in range(NK):
                    ge, go = 2 * k, 2 * k + 1
                    if 2 * k + 4 < NG:
                        load_xg(2 * k + 4)
                        load_xg(2 * k + 5)
                    if k == 5:
                        nc.sync.dma_start(Yf[0:32], y_d[0:32])
                    ps = [pdw.tile([128, 2, 512], F32, tag=f"tq{tq}",
                                   name=f"ps{k}_{tq}")
                          for tq in range(4)]
                    ya = yapool.tile([KP, WT, NCOL], F16, tag="ya",
                                     name=f"ya{k}")
                    for t in range(WT):
                        for dz in range(3):
                            for half, g in ((0, ge), (1, go)):
                                nc.tensor.matmul(
                                    ps[t // 2][64 * half:64 * half + MPAD,
                                               t % 2, 0:NCOL],
                                    wt_sb[:, g, dz, :],
                                    xg[g][:, t, dz:dz + DPC, :],
                                    start=(dz == 0), stop=(dz == 2),
                                    tile_position=(0, 64 * half),
                                )
                        if t % 2 == 1:
                            tq = t // 2
                            dst = ya[:, t - 1:t + 1, :]
                            src = ps[tq][:, :, 0:NCOL]
                            if tq % 2 == 0:
                                nc.scalar.activation(dst, src, RELU,
                                                     bias=sb1_sb[:, k:k + 1])
                            else:
                                nc.vector.tensor_scalar(dst, src,
                                                        sb1_sb[:, k:k + 1],
                                                        0.0, op0=ADD, op1=MAX)
                    # regroup via DRAM: SBUF->SBUF DMA is capped ~50GB/s,
                    # but SBUF->DRAM->SBUF runs at full ring rate; the load
                    # streams back per supergroup so the PW phase never waits
                    nc.scalar.dma_start(y_d[8 * k:8 * k + 8],
                                        ya.rearrange("m t r -> m (t r)"))

            # ---- pointwise phase ----
            nc.scalar.dma_start(Yf[32:48], y_d[32:48])
            nc.sync.dma_start(Yf[48:64], y_d[48:64])
            Yv = Y[:, 0:12].rearrange("c q t d j -> c (q t d j)")
            with tc.tile_pool(name="pspw", bufs=2, space="PSUM") as ppw:
                for q in range(NQ):
                    psz = [ppw.tile([F, 2, 512], F32, tag=tg,
                                    name=f"pz{tg}{q}")
                           for tg in ("pzA", "pzB")]
                    zt = zpool.tile([F, 4, QC], F16, tag="zt", name=f"zt{q}")
                    for s in range(4):
                        c0 = q * QP + s * QC
                        nc.tensor.matmul(psz[s // 2][:, s % 2, 0:QC],
                                         pw_sb[:], Yv[:, c0:c0 + QC],
                                         start=True, stop=True)
                    nc.scalar.activation(zt[:, 0:2, :], psz[0][:, :, 0:QC],
                                         RELU, bias=sb2_sb[:, 0:1])
                    nc.vector.tensor_scalar(zt[:, 2:4, :],
                                            psz[1][:, :, 0:QC],
                                            sb2_sb[:, 0:1], 0.0,
                                            op0=ADD, op1=MAX)
                    nc.sync.dma_start(
                        z_d[:, q * QP:(q + 1) * QP],
                        zt[:].rearrange("f s r -> f (s r)"))

    nc.compile()
    return nc


def _prep_inputs(x, dw_kernel, dw_bias, bn1_gamma, bn1_beta, bn1_mean,
                 bn1_var, pw_kernel, pw_bias, bn2_gamma, bn2_beta, bn2_mean,
                 bn2_var):
    """Build per-core input maps (numpy only, off the device clock)."""
    x = np.asarray(x, np.float32)
    dw = np.asarray(dw_kernel, np.float32)[:, :, :, 0, :]     # [3,3,3,C]
    a1 = np.asarray(bn1_gamma, np.float32) / np.sqrt(
        np.asarray(bn1_var, np.float32) + EPS)
    c1 = a1 * (np.asarray(dw_bias, np.float32)
               - np.asarray(bn1_mean, np.float32)) \
        + np.asarray(bn1_beta, np.float32)
    a2 = np.asarray(bn2_gamma, np.float32) / np.sqrt(
        np.asarray(bn2_var, np.float32) + EPS)
    c2 = a2 * (np.asarray(pw_bias, np.float32)
               - np.asarray(bn2_mean, np.float32)) \
        + np.asarray(bn2_beta, np.float32)

    # parity-folded depthwise stationaries [128, 16, 3, 64] (16 pad cols)
    wt = np.zeros((KP, NG, 3, MPAD), np.float32)
    dwa = dw * a1[None, None, None, :]
    dwg = dwa.reshape(3, 3, 3, NG, CG)                        # [dz,dy,dx,g,ci]
    for ci in range(CG):
        for wo in range(WO):
            for dx in range(3):
                for p in range(2):
                    for dy in range(3):
                        wt[ci * 32 + (wo + dx) * 4 + (p + dy), :, :,
                           ci * 16 + wo * 2 + p] = dwg[:, dy, dx, :, ci].T
    wt = wt.astype(np.float16)

    # DW bias per supergroup in act-partition order [128, 8]
    sb1 = np.zeros((128, NK), np.float32)
    for k in range(NK):
        for half in range(2):
            for ci in range(CG):
                ch = 8 * k + CG * half + ci
                lo = 64 * half + ci * 16
                sb1[lo:lo + 12, k] = c1[ch]
    sb2 = (c2[:, None]).astype(np.float32)

    pwk = (np.asarray(pw_kernel, np.float32)
           * a2[None, :]).astype(np.float16)

    xp = np.zeros((B, D + 2, H + 2, W + 2, C), np.float32)
    xp[:, 1:-1, 1:-1, 1:-1, :] = x

    hidx = 2 * np.arange(NJ)[:, None] + np.arange(4)[None, :]   # [24,4]
    widx = WO * np.arange(WT)[:, None] + np.arange(WI)[None, :]  # [8,8]

    in_maps = []
    for core in range(N_CORES):
        slab = core * DPC
        b, d0 = slab // D, slab % D
        sl = xp[b, d0:d0 + DI]                        # [14, 50, 50, C]
        xc = sl.transpose(3, 0, 1, 2)                 # [C, 14, 50, 50]
        xv = xc[:, :, hidx, :]                        # [C, 14, 24, 4, 50]
        xv = xv[..., widx]                            # [C,14,24,4,8,8]
        xt = np.ascontiguousarray(xv.transpose(0, 5, 3, 4, 1, 2)) \
            .reshape(NG, KP, WT, DI, NJ)
        in_maps.append({
            "xt": xt.astype(np.float16),
            "wt": wt, "pwk": pwk, "sb1": sb1, "sb2": sb2,
        })
    return in_maps


def _gather_output(results):
    z = np.empty((B, D, H, W, F), np.float32)
    for core in range(N_CORES):
        slab = core * DPC
        b, d0 = slab // D, slab % D
        zc = results[core]["z"].astype(np.float32)    # [F, NPOS]
        zc = zc.reshape(F, WO, 2, WT, DPC, NJ).transpose(4, 5, 2, 3, 1, 0)
        z[b, d0:d0 + DPC] = zc.reshape(DPC, H, W, F)
    return z


def kernel(**inputs):
    global _COMPILED
    if _COMPILED is None:
        _COMPILED = _build_bass()
    in_maps = _prep_inputs(**inputs)
    res = run_bass_kernel_spmd(_COMPILED, in_maps,
                               core_ids=list(range(N_CORES)))
    return _gather_output(res.results)


if __name__ == "__main__":
    pass


# revision 18
# speedup vs baseline: 1.0995x; 1.0995x over previous
"""Trainium2 Bass kernel for DepthwiseSeparableConv3d (inference).

Problem: x[2,48,48,48,64] -> dw3x3x3 depthwise + BN + ReLU -> 1x1x1 conv
(64->128) + BN + ReLU -> z[2,48,48,48,128], all f32.

Strategy (8 NeuronCores, data-parallel over (b,d) slabs, 12 slabs/core):
 - Host pre-pads D/H/W and folds the BN scales into the conv weights;
   BN biases ride along in the ReLU stage (ScalarE activation bias /
   DVE tensor_scalar add+max), so the device does matmul+ReLU+DMA only.
 - Depthwise conv on TensorE as a parity-folded 2D block-Toeplitz:
   K = 128 partitions = (4 ch x 8 w-in x 4 h-parity),
   M = 48 partitions  = (4 ch x 6 w-out x 2 h-parity).
   The 3 w-taps AND the 3 h-taps live in the Toeplitz stationary
   (partition (ci,wi,hw) holds column hp=2j+hw of the padded image, so
   every dy tap is reachable without a free-dim shift); only the 3
   dz-taps are PSUM-accumulated pumps.  9 -> 3 pumps per position vs a
   1-D Toeplitz: 1.5x fewer PE rows.
 - Two groups share each PSUM bank via PE column tiling
   (tile_position (0,0)/(0,64)); ReLU+bias alternates ScalarE/DVE.
 - The (ci,wo,p)->channel regroup DMA has 4-partition destinations and
   is split across the SP and ACT HWDGE rings.
 - Serial phases: all DW first (uninterrupted matmul stream keeps the
   PE at the top p-state), then the PW sweep.  A burst of dummy
   matmuls during the input-DMA wait pre-warms the PE clock.
 - Everything on-chip is fp16; output z is fp16, widened on host.
"""

import sys

for _p in ("/opt/trn_rl_repo", "/opt/pypackages"):
    if _p not in sys.path:
        sys.path.insert(0, _p)

import numpy as np

import concourse.bass as bass
import concourse.tile as tile
from concourse import bacc, mybir
from concourse.bass_utils import run_bass_kernel_spmd

# ----- problem constants (hardcoded per spec) -----
B, D, H, W, C, F = 2, 48, 48, 48, 64, 128
EPS = 1e-3
N_CORES = 8
DPC = (B * D) // N_CORES          # 12 d-slabs per core
DI = DPC + 2                      # 14 with halo
CG = 4                            # channels per group
NG = C // CG                      # 16 groups
NK = NG // 2                      # 8 supergroups (2 groups PSUM-stacked)
WT = 8                            # w tiles
WO = 6                            # w outputs per tile
WI = 8                            # w inputs per tile
NJ = 24                           # h-pairs
KP = CG * WI * 4                  # 128 partitions (ci, wi, hw)
MP = CG * WO * 2                  # 48 real outputs (ci, wo, p)
MPAD = 64                         # stationary cols padded to a PE col-tile
NCOL = DPC * NJ                   # 288 columns per pump
NPOS = WT * WO * 2 * DPC * NJ     # 27648 positions per core
NQ = 18                           # PW macro-chunks
QP = NPOS // NQ                   # 1536 positions per macro-chunk
QC = QP // 4                      # 384 per pump
WARMUP_MM = 48                    # dummy matmuls to pre-ramp the PE clock

F16 = mybir.dt.float16
F32 = mybir.dt.float32

_COMPILED = None


def _build_bass():
    nc = bacc.Bacc("TRN2", target_bir_lowering=False, debug=False,
                   num_devices=N_CORES)

    xt_d = nc.dram_tensor("xt", [NG, KP, WT, DI, NJ], F16,
                          kind="ExternalInput").ap()
    wt_d = nc.dram_tensor("wt", [KP, NG, 3, MPAD], F16,
                          kind="ExternalInput").ap()
    pw_d = nc.dram_tensor("pwk", [C, F], F16, kind="ExternalInput").ap()
    sb1_d = nc.dram_tensor("sb1", [128, NK], F32, kind="ExternalInput").ap()
    sb2_d = nc.dram_tensor("sb2", [F, 1], F32, kind="ExternalInput").ap()
    z_d = nc.dram_tensor("z", [F, NPOS], F16, kind="ExternalOutput").ap()
    y_d = nc.dram_tensor("ydram", [C, 16 * WT * DPC * NJ], F16,
                         kind="Internal").ap()

    RELU = mybir.ActivationFunctionType.Relu
    ADD = mybir.AluOpType.add
    MAX = mybir.AluOpType.max

    with tile.TileContext(nc) as tc:
        with (
            tc.tile_pool(name="consts", bufs=1) as consts,
            tc.tile_pool(name="xg", bufs=NG) as xpool,
            tc.tile_pool(name="ya", bufs=6) as yapool,
            tc.tile_pool(name="Ybig", bufs=1) as Ypool,
            tc.tile_pool(name="zt", bufs=3) as zpool,
        ):
            wt_sb = consts.tile([KP, NG, 3, MPAD], F16)
            pw_sb = consts.tile([C, F], F16)
            sb1_sb = consts.tile([128, NK], F32)
            sb2_sb = consts.tile([F, 1], F32)
            Y = Ypool.tile([C, 16, WT, DPC, NJ], F16)
            Yf = Y.rearrange("c q t d j -> c (q t d j)")
            xg = [xpool.tile([KP, WT, DI, NJ], F16, tag="xg", name=f"xg{g}")
                  for g in range(NG)]

            # Need-ordered input stream.  Only a few xg DMAs may be in
            # flight at once: descriptors of concurrent DMAs round-robin
            # across the DMA engines, so a deep queue makes every tile
            # finish late; the rest are issued from inside the k-loop.
            nc.sync.dma_start(wt_sb[:, 0:2], wt_d[:, 0:2])
            nc.sync.dma_start(xg[0][:], xt_d[0])
            nc.sync.dma_start(xg[1][:], xt_d[1])
            nc.sync.dma_start(sb1_sb[:], sb1_d[:])
            nc.sync.dma_start(xg[2][:], xt_d[2])
            nc.sync.dma_start(xg[3][:], xt_d[3])
            nc.sync.dma_start(wt_sb[:, 2:NG], wt_d[:, 2:NG])
            nc.sync.dma_start(pw_sb[:], pw_d[:])
            nc.sync.dma_start(sb2_sb[:], sb2_d[:])
            nc.sync.dma_start(xg[4][:], xt_d[4])
            nc.sync.dma_start(xg[5][:], xt_d[5])
            nc.sync.dma_start(xg[6][:], xt_d[6])
            nc.sync.dma_start(xg[7][:], xt_d[7])

            with tc.tile_pool(name="psdw", bufs=1, space="PSUM") as pdw:
                # p-state warmup: pump the PE on already-loaded weights while
                # the first xg DMAs are in flight; results are never read.
                if WARMUP_MM:
                    wps = pdw.tile([128, 2, 512], F32, tag="tq0", name="warm")
                    wrhs = wt_sb[:, 0:2, :, :].rearrange("k a b m -> k (a b m)")
                    for i in range(WARMUP_MM):
                        nc.tensor.matmul(wps[0:MPAD, i % 2, 0:NCOL],
                                         wt_sb[:, 0, 0, :], wrhs[:, 0:NCOL],
                                         start=True, stop=True,
                                         skip_group_check=True)

                for k in range(NK):
                    ge, go = 2 * k, 2 * k + 1
                    if 2 * k + 8 < NG:
                        nc.sync.dma_start(xg[2 * k + 8][:], xt_d[2 * k + 8])
                        nc.sync.dma_start(xg[2 * k + 9][:], xt_d[2 * k + 9])
                    ps = [pdw.tile([128, 2, 512], F32, tag=f"tq{tq}",
                                   name=f"ps{k}_{tq}")
                          for tq in range(4)]
                    ya = yapool.tile([KP, WT, NCOL], F16, tag="ya",
                                     name=f"ya{k}")
                    for t in range(WT):
                        for dz in range(3):
                            for half, g in ((0, ge), (1, go)):
                                nc.tensor.matmul(
                                    ps[t // 2][64 * half:64 * half + MPAD,
                                               t % 2, 0:NCOL],
                                    wt_sb[:, g, dz, :],
                                    xg[g][:, t, dz:dz + DPC, :],
                                    start=(dz == 0), stop=(dz == 2),
                                    tile_position=(0, 64 * half),
                                )
                        if t % 2 == 1:
                            tq = t // 2
                            dst = ya[:, t - 1:t + 1, :]
                            src = ps[tq][:, :, 0:NCOL]
                            if tq % 2 == 0:
                                nc.scalar.activation(dst, src, RELU,
                                                     bias=sb1_sb[:, k:k + 1])
                            else:
                                nc.vector.tensor_scalar(dst, src,
                                                        sb1_sb[:, k:k + 1],
                                                        0.0, op0=ADD, op1=MAX)
                    # regroup via DRAM: SBUF->SBUF DMA is capped ~50GB/s,
                    # but SBUF->DRAM->SBUF runs at full ring rate; the load
                    # streams back per supergroup so the PW phase never waits
                    nc.scalar.dma_start(y_d[8 * k:8 * k + 8],
                                        ya.rearrange("m t r -> m (t r)"))

            # ---- pointwise phase ----
            nc.sync.dma_start(Yf[0:32], y_d[0:32])
            nc.scalar.dma_start(Yf[32:64], y_d[32:64])
            Yv = Y[:, 0:12].rearrange("c q t d j -> c (q t d j)")
            with tc.tile_pool(name="pspw", bufs=2, space="PSUM") as ppw:
                for q in range(NQ):
                    psz = [ppw.tile([F, 2, 512], F32, tag=tg,
                                    name=f"pz{tg}{q}")
                           for tg in ("pzA", "pzB")]
                    zt = zpool.tile([F, 4, QC], F16, tag="zt", name=f"zt{q}")
                    for s in range(4):
                        c0 = q * QP + s * QC
                        nc.tensor.matmul(psz[s // 2][:, s % 2, 0:QC],
                                         pw_sb[:], Yv[:, c0:c0 + QC],
                                         start=True, stop=True)
                    nc.scalar.activation(zt[:, 0:2, :], psz[0][:, :, 0:QC],
                                         RELU, bias=sb2_sb[:, 0:1])
                    nc.vector.tensor_scalar(zt[:, 2:4, :],
                                            psz[1][:, :, 0:QC],
                                            sb2_sb[:, 0:1], 0.0,
                                            op0=ADD, op1=MAX)
                    nc.sync.dma_start(
                        z_d[:, q * QP:(q + 1) * QP],
                        zt[:].rearrange("f s r -> f (s r)"))

    nc.compile()
    return nc


def _prep_inputs(x, dw_kernel, dw_bias, bn1_gamma, bn1_beta, bn1_mean,
                 bn1_var, pw_kernel, pw_bias, bn2_gamma, bn2_beta, bn2_mean,
                 bn2_var):
    """Build per-core input maps (numpy only, off the device clock)."""
    x = np.asarray(x, np.float32)
    dw = np.asarray(dw_kernel, np.float32)[:, :, :, 0, :]     # [3,3,3,C]
    a1 = np.asarray(bn1_gamma, np.float32) / np.sqrt(
        np.asarray(bn1_var, np.float32) + EPS)
    c1 = a1 * (np.asarray(dw_bias, np.float32)
               - np.asarray(bn1_mean, np.float32)) \
        + np.asarray(bn1_beta, np.float32)
    a2 = np.asarray(bn2_gamma, np.float32) / np.sqrt(
        np.asarray(bn2_var, np.float32) + EPS)
    c2 = a2 * (np.asarray(pw_bias, np.float32)
               - np.asarray(bn2_mean, np.float32)) \
        + np.asarray(bn2_beta, np.float32)

    # parity-folded depthwise stationaries [128, 16, 3, 64] (16 pad cols)
    wt = np.zeros((KP, NG, 3, MPAD), np.float32)
    dwa = dw * a1[None, None, None, :]
    dwg = dwa.reshape(3, 3, 3, NG, CG)                        # [dz,dy,dx,g,ci]
    for ci in range(CG):
        for wo in range(WO):
            for dx in range(3):
                for p in range(2):
                    for dy in range(3):
                        wt[ci * 32 + (wo + dx) * 4 + (p + dy), :, :,
                           ci * 16 + wo * 2 + p] = dwg[:, dy, dx, :, ci].T
    wt = wt.astype(np.float16)

    # DW bias per supergroup in act-partition order [128, 8]
    sb1 = np.zeros((128, NK), np.float32)
    for k in range(NK):
        for half in range(2):
            for ci in range(CG):
                ch = 8 * k + CG * half + ci
                lo = 64 * half + ci * 16
                sb1[lo:lo + 12, k] = c1[ch]
    sb2 = (c2[:, None]).astype(np.float32)

    pwk = (np.asarray(pw_kernel, np.float32)
           * a2[None, :]).astype(np.float16)

    xp = np.zeros((B, D + 2, H + 2, W + 2, C), np.float32)
    xp[:, 1:-1, 1:-1, 1:-1, :] = x

    hidx = 2 * np.arange(NJ)[:, None] + np.arange(4)[None, :]   # [24,4]
    widx = WO * np.arange(WT)[:, None] + np.arange(WI)[None, :]  # [8,8]

    in_maps = []
    for core in range(N_CORES):
        slab = core * DPC
        b, d0 = slab // D, slab % D
        sl = xp[b, d0:d0 + DI]                        # [14, 50, 50, C]
        xc = sl.transpose(3, 0, 1, 2)                 # [C, 14, 50, 50]
        xv = xc[:, :, hidx, :]                        # [C, 14, 24, 4, 50]
        xv = xv[..., widx]                            # [C,14,24,4,8,8]
        xt = np.ascontiguousarray(xv.transpose(0, 5, 3, 4, 1, 2)) \
            .reshape(NG, KP, WT, DI, NJ)
        in_maps.append({
            "xt": xt.astype(np.float16),
            "wt": wt, "pwk": pwk, "sb1": sb1, "sb2": sb2,
        })
    return in_maps


def _gather_output(results):
    z = np.empty((B, D, H, W, F), np.float32)
    for core in range(N_CORES):
        slab = core * DPC
        b, d0 = slab // D, slab % D
        zc = results[core]["z"].astype(np.float32)    # [F, NPOS]
        zc = zc.reshape(F, WO, 2, WT, DPC, NJ).transpose(4, 5, 2, 3, 1, 0)
        z[b, d0:d0 + DPC] = zc.reshape(DPC, H, W, F)
    return z


def kernel(**inputs):
    global _COMPILED
    if _COMPILED is None:
        _COMPILED = _build_bass()
    in_maps = _prep_inputs(**inputs)
    res = run_bass_kernel_spmd(_COMPILED, in_maps,
                               core_ids=list(range(N_CORES)))
    return _gather_output(res.results)


if __name__ == "__main__":
    pass
